# revision 3
# baseline (speedup 1.0000x reference)
"""Trainium2 Bass kernel v4: row-softmax + embedding gather, all-quad fp16.

Structure (calibrated on HW traces of baseline/v2/v3):
  - GpSimd desc-gen is the wall (~7-9ns/desc, 25600 read descs/core).
    Normal-mode dma_gather (engine-held) with 3-queue rotation overlaps
    drains under gen (baseline-proven ~4.3-4.75us/op); prepare_only
    turned out strictly worse (gen +1.4us/op doorbell overhead).
  - fp16 quads (1536B/desc) halve drain vs f32 and keep rel err ~1e-3.
  - Sub-row select: 3 predicated copies IN PLACE into the quad's q=0
    lane (DVE), then per-position exp on ACT reads the strided lane,
    writes a compact S tile, and accumulates the row sum in one op.
  - Normalize: DVE reciprocal + DVE broadcast multiply.
  - Output fp16, one big contiguous write per 20-position super-chunk;
    host upcasts to f32.
"""

import sys

import numpy as np

sys.path.insert(0, "/opt/trn_rl_repo")

N_POI = 100000
N_BINS = 168
DPB = 192  # padded row length in fp16 elems (384B)
NQ = N_POI // 4
BATCH = 1024
SEQ = 200
N_CORES = 8
BPC = BATCH // N_CORES

M = 5  # seq positions per quad dma_gather op (HW ring limit: 640 descs OK)
SC = 4  # gathers per compute super-chunk (20 positions)

_NC_CACHE = {}


def build(seq=SEQ, m=M, sc=SC, nqueues=4, tbufs=3):
    import concourse.bacc as bacc
    import concourse.tile as tile
    from concourse import bass, mybir

    sup = m * sc
    assert seq % sup == 0
    nsup = seq // sup
    nidx = BPC * m
    nc = bacc.Bacc(
        "TRN2",
        target_bir_lowering=False,
        debug=False,
        enable_asserts=False,
        num_devices=N_CORES,
        num_swdge_queues=nqueues,
        dynamic_dma_scratch_size=65536,
        enable_partition_id=False,
    )
    qtab = nc.dram_tensor(
        "qtab", [NQ, 4 * DPB], mybir.dt.float16, kind="ExternalInput"
    ).ap()
    widx = nc.dram_tensor(
        "widx", [128, seq * 8], mybir.dt.int16, kind="ExternalInput"
    ).ap()
    msk = nc.dram_tensor(
        "msk", [BPC, 3 * seq], mybir.dt.uint8, kind="ExternalInput"
    ).ap()
    out = nc.dram_tensor(
        "out", [BPC, seq, N_BINS], mybir.dt.float16, kind="ExternalOutput"
    ).ap()

    with tile.TileContext(nc) as tc, nc.allow_low_precision(
        reason="fp16 softmax; rel-err budget is 2e-2"
    ):
        with tc.tile_pool(name="const", bufs=1) as cpool, tc.tile_pool(
            name="big", bufs=tbufs
        ) as tpool, tc.tile_pool(name="sel", bufs=tbufs + 1) as spool, tc.tile_pool(
            name="small", bufs=2 * tbufs
        ) as smpool:
            wt = cpool.tile([128, seq * 8], mybir.dt.int16)
            nc.sync.dma_start(out=wt[:], in_=widx[:])
            mt = cpool.tile([BPC, 3 * seq], mybir.dt.uint8)
            nc.sync.dma_start(out=mt[:], in_=msk[:])
            m3 = mt[:].rearrange("p (q s) -> p q s", q=3)

            pending = None  # (S, S3, sums, s) awaiting normalize+write

            def flush(pend):
                S_, S3_, sums_, ps = pend
                rec = smpool.tile([BPC, sup], mybir.dt.float16, tag="rec")
                nc.vector.reciprocal(out=rec[:], in_=sums_[:])
                nc.vector.tensor_tensor(
                    out=S3_,
                    in0=S3_,
                    in1=rec[:].to_broadcast([BPC, sup, N_BINS]),
                    op=mybir.AluOpType.mult,
                )
                nc.sync.dma_start(
                    out=out[:, ps * sup : (ps + 1) * sup, :], in_=S_[:]
                )

            for s in range(nsup):
                T = tpool.tile([BPC, sup * 4 * DPB], mybir.dt.float16, tag="T")
                T3 = T[:].rearrange("p (u d) -> p u d", u=sup)
                T4 = T[:].rearrange("p (u q d) -> p u q d", u=sup, q=4)
                for g in range(sc):
                    c = s * sc + g
                    nc.gpsimd.dma_gather(
                        out_ap=T3[:, g * m : (g + 1) * m, :],
                        in_ap=qtab[:],
                        idxs_ap=wt[:, c * m * 8 : (c + 1) * m * 8],
                        num_idxs=nidx,
                        num_idxs_reg=nidx,
                        elem_size=4 * DPB,
                        elem_step=4 * DPB,
                        single_packet=True,
                        queue_num=1 + c % (nqueues - 1),
                    )

                S = spool.tile([BPC, sup * N_BINS], mybir.dt.float16, tag="S")
                S3 = S[:].rearrange("p (u d) -> p u d", u=sup)
                sums = smpool.tile([BPC, sup], mybir.dt.float32, tag="sums")
                half = sup // 2
                for h in range(2):
                    lo, hi = h * half, (h + 1) * half
                    # select sub-row in place into T's q=0 lane (half-super)
                    for qq in (1, 2, 3):
                        nc.vector.copy_predicated(
                            out=T4[:, lo:hi, 0, :N_BINS],
                            mask=m3[
                                :, qq - 1, s * sup + lo : s * sup + hi
                            ].to_broadcast([BPC, half, N_BINS]),
                            data=T4[:, lo:hi, qq, :N_BINS],
                        )
                    if h == 1 and pending is not None:
                        # normalize+write the previous super while ACT works
                        flush(pending)
                    for u in range(lo, hi):
                        nc.scalar.activation(
                            out=S3[:, u, :],
                            in_=T4[:, u, 0, :N_BINS],
                            func=mybir.ActivationFunctionType.Exp,
                            accum_out=sums[:, u : u + 1],
                        )
                pending = (S, S3, sums, s)
            flush(pending)
    nc.compile()
    return nc


def _prep_inputs(wekn, table, m=M):
    qt = np.zeros((NQ, 4, DPB), dtype=np.float16)
    qt[:, :, :N_BINS] = table.reshape(NQ, 4, N_BINS).astype(np.float16)
    qt = np.ascontiguousarray(qt.reshape(NQ, 4 * DPB))
    nch = SEQ // m
    in_maps = []
    for core in range(N_CORES):
        wc = wekn[core * BPC : (core + 1) * BPC]
        quad = (wc // 4).astype(np.int16)
        sub = wc % 4
        wi = np.empty((16, SEQ * 8), dtype=np.int16)
        for c in range(nch):
            walk = quad[:, c * m : (c + 1) * m].T.reshape(-1)
            wi[:, c * m * 8 : (c + 1) * m * 8] = walk.reshape(m * 8, 16).T
        msk = np.empty((BPC, 3, SEQ), dtype=np.uint8)
        for q in (1, 2, 3):
            msk[:, q - 1] = (sub == q).astype(np.uint8)
        in_maps.append(
            {
                "qtab": qt,
                "widx": np.tile(wi, (8, 1)),
                "msk": np.ascontiguousarray(msk.reshape(BPC, 3 * SEQ)),
            }
        )
    return in_maps


def _get_nc():
    if "nc" not in _NC_CACHE:
        _NC_CACHE["nc"] = build()
    return _NC_CACHE["nc"]


def kernel(**inputs) -> np.ndarray:
    wekn = np.asarray(inputs["inputs_wekn"]).astype(np.int64)
    table = np.ascontiguousarray(
        np.asarray(inputs["poi_freq_matrix"], dtype=np.float32)
    )
    assert wekn.shape == (BATCH, SEQ) and table.shape == (N_POI, N_BINS)

    from concourse.bass_utils import run_bass_kernel_spmd

    nc = _get_nc()
    in_maps = _prep_inputs(wekn, table)
    res = run_bass_kernel_spmd(nc, in_maps, core_ids=list(range(N_CORES)))
    return np.concatenate(
        [
            np.asarray(res.results[c]["out"], dtype=np.float32)
            for c in range(N_CORES)
        ],
        axis=0,
    )


if __name__ == "__main__":
    rng = np.random.default_rng(0)
    inputs = {
        "venueid2coor": rng.random((N_POI, 2), dtype=np.float32),
        "inputs_wekn": rng.integers(0, N_POI, size=(BATCH, SEQ), dtype=np.int64),
        "poi_freq_matrix": rng.standard_normal((N_POI, N_BINS), dtype=np.float32),
    }
    out = kernel(**inputs)
    print(out.shape, out.dtype)
